# revision 26
# baseline (speedup 1.0000x reference)
"""Trainium2 Bass kernel for nn_BasicBlock (binary-activation conv block).

Reference forward (per element):
    act  = sign(x + b0)                      # {-1, 0, +1}
    bw   = scale_c * sign(w),  scale_c = mean|w| over (ci,kh,kw)
    raw  = conv3x3(act, sign(w))             # exact small integers
    y    = (scale*raw - mu) * rsqrt(var + eps) * gamma + beta + x + b1
    out  = prelu(y, alpha) + b2
with BN stats (mu, var) over the FULL batch (sync-BN across cores).

Strategy (8 NeuronCores, batch-sharded 4 imgs/core):
  - act/weights are +-1 in fp8e4 -> matmuls with fp32 PSUM accumulation are
    EXACT.  conv = 9 shifted matmuls (K=64, M=64); FOUR streams run
    concurrently in the 128x128 PE array quadrants via tile_position.
  - SINGLE conv pass: each chunk's psum is copied to SBUF as fp16 (raw
    values are integers |raw| <= 576 -> exact in fp16).  The copy also
    accumulates the per-partition SUM (accum_out); GpSimd computes the
    per-partition SUMSQ from the fp16 copy.  No conv recompute, no
    bn_stats bottleneck on DVE.
  - sync-BN: a tiny warm-up AllGather at t~0 absorbs core-launch skew and
    CC stream spin-up; the real stats exchange is one small AllGather of
    per-channel (sum, sumsq) + local reduce.
  - epilogue (post-stats, tensor-free):
        t   = A*raw + x          (scalar_tensor_tensor; DVE and GpSimd
                                  alternate chunks)
        out = Prelu(t + B)       (one ACT pass, per-channel alpha)
    where A = gamma*scale*rsqrt(var+eps), B = beta + b1 - mu*A.
  - x streamed in 8 depth-2-chained DMA blocks so act bands become
    available early while the tail still streams at full HBM bandwidth.

kernel(**inputs) takes FULL inputs, shards, runs SPMD on cores 0-7, gathers.
"""
import os
import numpy as np
from contextlib import ExitStack

from concourse import bacc, mybir, tile
from concourse.tile_rust import add_dep_helper
from concourse.bass_utils import run_bass_kernel_spmd

# ---------------- problem constants (hardcoded per spec) ----------------
N_CORES = 8
IMGS = 4          # images per core
C = 64            # channels
H = W = 112
HP = WP = 114     # zero-padded act dims
BN_EPS = 1e-5
NG = 32 * H * W   # global BN count per channel

f32 = mybir.dt.float32
f16 = mybir.dt.float16
fp8 = mybir.dt.float8e4

RPC = 4            # output rows per psum bank
NCHUNK = H // RPC  # 28 row-chunks
GRP = 2            # chunks per staged output tile

# x stream blocks: 8 DMAs total (2 per block) so the 8 DMA-completion
# semaphore lanes never throttle the stream; a small first block gets the
# first act band ready early
XBLK = [(0, 15), (15, 44), (44, 85), (85, 112)]


def build_program(with_b0: bool, with_b2: bool, sync_bn: bool):
    nc = bacc.Bacc("TRN2", target_bir_lowering=False, debug=False,
                   num_devices=N_CORES)

    x_d = nc.dram_tensor("x", [IMGS, C, H, W], f32, kind="ExternalInput")
    b0_d = nc.dram_tensor("b0", [1, C, 1, 1], f32, kind="ExternalInput")
    w_d = nc.dram_tensor("w", [C, C, 3, 3], f32, kind="ExternalInput")
    gamma_d = nc.dram_tensor("gamma", [C], f32, kind="ExternalInput")
    beta_d = nc.dram_tensor("beta", [C], f32, kind="ExternalInput")
    b1_d = nc.dram_tensor("b1", [1, C, 1, 1], f32, kind="ExternalInput")
    alpha_d = nc.dram_tensor("alpha", [C], f32, kind="ExternalInput")
    b2_d = nc.dram_tensor("b2", [1, C, 1, 1], f32, kind="ExternalInput")
    # host-marshalled transposed weights: wt[i, t, o] = w[o, i, kh, kw]
    wt_d = nc.dram_tensor("wt", [C, 9, C], f32, kind="ExternalInput")
    out_d = nc.dram_tensor("out", [IMGS, C, H, W], f32, kind="ExternalOutput")

    AF = mybir.ActivationFunctionType
    OP = mybir.AluOpType

    with tile.TileContext(nc) as tc, ExitStack() as ctx:
        pool = ctx.enter_context(tc.tile_pool(name="sbuf", bufs=1))
        actp = ctx.enter_context(tc.tile_pool(name="actp", bufs=3))
        stgp = ctx.enter_context(tc.tile_pool(name="stgp", bufs=2))
        outp = ctx.enter_context(tc.tile_pool(name="outp", bufs=3))
        psum = ctx.enter_context(
            tc.tile_pool(name="psum", bufs=4, space="PSUM"))
        dram = ctx.enter_context(tc.tile_pool(name="dram", bufs=1, space="DRAM"))

        # collective warm-up: absorbs core-launch skew + CC stream spin-up
        warm_sb = pool.tile([8, 4], f32)
        nc.gpsimd.memset(warm_sb[:], 0.0)
        warm_in = dram.tile([8, 4], f32)
        warm_out = dram.tile([64, 4], f32)
        nc.scalar.dma_start(warm_in[:], warm_sb[:])
        nc.gpsimd.collective_compute(
            "AllGather", OP.bypass, ins=[warm_in.opt()], outs=[warm_out.opt()],
            replica_groups=[list(range(N_CORES))])

        # ------------- transposed weights (gpsimd queue, ahead of x) ----
        wt_f = pool.tile([64, 9, 64], f32)
        nc.gpsimd.dma_start(wt_f[:], wt_d.ap().rearrange("i t o -> i (t o)")
                            .rearrange("i (t o) -> i t o", t=9))

        def load_params():
            """Small params; scalar-queue DMAs + DVE math.  Issued late
            (before stats) unless sign needs b0."""
            par = pool.tile([64, 6], f32)
            nc.sync.dma_start(par[:, 0:1], b0_d.ap().rearrange("a c e f -> (a c) (e f)"))
            nc.sync.dma_start(par[:, 1:2], gamma_d.ap().rearrange("c -> c ()"))
            nc.sync.dma_start(par[:, 2:3], beta_d.ap().rearrange("c -> c ()"))
            nc.sync.dma_start(par[:, 3:4], b1_d.ap().rearrange("a c e f -> (a c) (e f)"))
            nc.sync.dma_start(par[:, 4:5], alpha_d.ap().rearrange("c -> c ()"))
            nc.sync.dma_start(par[:, 5:6], b2_d.ap().rearrange("a c e f -> (a c) (e f)"))
            rep = pool.tile([128, 6], f32)
            nc.vector.tensor_copy(rep[0:64, :], par[:])
            nc.gpsimd.dma_start(rep[64:128, :], rep[0:64, :])
            return rep

        # ---------------- x load: 8 blocks, depth-2 chained -------------
        x_sb = pool.tile([128, 2, H, W], f32)
        x_v = x_d.ap().rearrange("i c h w -> (i c) h w")
        blk_dmas = []
        for k, (r0, r1) in enumerate(XBLK):
            cur = []
            for s in range(2):
                src = x_v[128 * s:128 * (s + 1), r0:r1, :]
                ins = nc.sync.dma_start(x_sb[:, s, r0:r1, :], src)
                cur.append(ins)
            blk_dmas.append(cur)

        # sgn of transposed weights -> fp8 (+-1 exact); conv needs only this
        with tc.high_priority():
            nc.vector.tensor_scalar(wt_f[:], wt_f[:], 0.0, None, op0=OP.is_gt)
            w_taps = pool.tile([128, 9, 64], fp8)
            nc.vector.tensor_scalar(w_taps[0:64, :, :], wt_f[:], 2.0, -1.0,
                                    op0=OP.mult, op1=OP.add)
            nc.gpsimd.dma_start(w_taps[64:128, :, :], w_taps[0:64, :, :])
        # |w| scale (only needed pre-stats; replicate deferred to the
        # late-params section so it never blocks the gpsimd act swaps)
        w_sb = pool.tile([64, 576], f32)
        nc.scalar.dma_start(w_sb[:], w_d.ap().rearrange("o i kh kw -> o (i kh kw)"))
        scale128 = pool.tile([128, 1], f32)
        nc.vector.tensor_reduce(scale128[0:64, :], w_sb[:], axis=mybir.AxisListType.X,
                                op=OP.add, apply_absolute_value=True)
        nc.vector.tensor_scalar(scale128[0:64, :], scale128[0:64, :], 1.0 / 576.0,
                                None, op0=OP.mult)

        rep = load_params() if with_b0 else None
        if rep is not None:
            b0_ap = rep[:, 0:1]

        # ---------------- raw / stats storage ----------------
        raw_t = pool.tile([128, 2, NCHUNK, 448], f16)
        sums_t = pool.tile([128, NCHUNK], f32)
        ssqs_t = pool.tile([128, NCHUNK], f32)
        junk = pool.tile([128, 2, 448], f32)
        junk2 = pool.tile([128, 2, 448], f32)


        # 4-quadrant interleave order per tap:
        #   (slot, act_half_base, tile_position, psum_base)
        QORDER = [
            (1, 0, (0, 64), 64),    # img3
            (0, 0, (0, 0), 0),      # img0
            (0, 64, (64, 64), 64),  # img1
            (1, 64, (64, 0), 0),    # img2
        ]

        sign_kw = dict(bias=rep[:, 0:1]) if with_b0 else {}

        def make_band(b):
            """Produce act band b: sign of padded rows [28b, 28b+30) in two
            units; slot1 half-swapped via a rotating stage tile."""
            ctx2 = ExitStack()
            ctx2.enter_context(tc.high_priority())
            ab_t = actp.tile([128, 2, 30, WP], fp8, tag="act")
            stg = stgp.tile([128, 30, WP], fp8, tag="stg")
            nc.gpsimd.memset(ab_t[:, 0, :, 0:WP:113], 0.0)
            nc.gpsimd.memset(stg[:, :, 0:WP:113], 0.0)
            if b == 0:
                nc.gpsimd.memset(ab_t[:, :, 0:1, :], 0.0)
            if b == 3:
                nc.gpsimd.memset(ab_t[:, :, 29:30, :], 0.0)
            lo = max(1, 28 * b)
            hi = min(113, 28 * b + 30)
            for (a0, a1) in ((lo, 28 * b + 16), (28 * b + 16, hi)):
                nr = a1 - a0
                l0 = a0 - 28 * b
                xr = a0 - 1
                nc.scalar.activation(ab_t[:, 0, l0:l0 + nr, 1:113],
                                     x_sb[:, 0, xr:xr + nr, :], AF.Sign,
                                     **sign_kw)
                nc.scalar.activation(stg[:, l0:l0 + nr, 1:113],
                                     x_sb[:, 1, xr:xr + nr, :], AF.Sign,
                                     **sign_kw)
                nc.scalar.dma_start(ab_t[64:128, 1, l0:l0 + nr, :],
                                    stg[0:64, l0:l0 + nr, :])
                nc.scalar.dma_start(ab_t[0:64, 1, l0:l0 + nr, :],
                                    stg[64:128, l0:l0 + nr, :])
            ctx2.close()
            return ab_t

        def conv_pair(pt, band, lrow):
            """9-tap binary conv for one row-chunk of BOTH slots into one
            [128, 2, 512] psum tile (bank s = slot s, cols 0:448 used),
            matmuls interleaved across all 4 PE quadrants."""
            for t in range(9):
                ky, kx = divmod(t, 3)
                for (s, ab, tp, pb) in QORDER:
                    rhs = band[ab:ab + 64, s, lrow + ky:lrow + ky + 4, kx:kx + 112]
                    dst = pt[pb:pb + 64, s, 0:448].rearrange(
                        "p (r c) -> p r c", r=4)
                    nc.tensor.matmul(
                        dst, w_taps[ab:ab + 64, t, :], rhs,
                        start=(t == 0), stop=(t == 8), tile_position=tp)

        # ------------- conv pass: psum -> fp16 raw + sum/sumsq ----------
        bands = [make_band(0)]
        for r in range(NCHUNK):
            b, l = divmod(r, 7)
            if l == 0 and b + 1 < 4:
                bands.append(make_band(b + 1))
            band, lrow = bands[b], 4 * l
            pt = psum.tile([128, 2, 512], f32, tag="cv")
            conv_pair(pt, band, lrow)
            # exact fp16 copy of the integer conv output; accumulates the
            # per-partition sum as a side effect
            nc.vector.tensor_scalar(raw_t[:, :, r, :], pt[:, :, 0:448],
                                    1.0, 0.0, op0=OP.mult, op1=OP.add,
                                    accum_out=sums_t[:, r:r + 1])
            # sumsq, odd chunks: fused DVE stt + accum (paces with the copy)
            if r % 2 == 1:
                nc.vector.scalar_tensor_tensor(junk[:], raw_t[:, :, r, :], 1.0,
                                               raw_t[:, :, r, :], op0=OP.mult,
                                               op1=OP.mult,
                                               accum_out=ssqs_t[:, r:r + 1])

        # sumsq, even chunks: ACT Square + accum, issued AFTER all sign work
        # so band production is never head-of-line blocked on the ACT queue
        for r in range(0, NCHUNK, 2):
            nc.scalar.activation(junk2[:], raw_t[:, :, r, :], AF.Square,
                                 accum_out=ssqs_t[:, r:r + 1])

        # ---------------- late params + stats-independent math ----------
        if rep is None:
            rep = load_params()
        nc.gpsimd.dma_start(scale128[64:128, :], scale128[0:64, :])
        gamma_ap = rep[:, 1:2]
        beta_ap = rep[:, 2:3]
        b1_ap = rep[:, 3:4]
        alpha_ap = rep[:, 4:5]
        b2_ap = rep[:, 5:6]
        sc2 = pool.tile([128, 1], f32)
        nc.vector.tensor_tensor(sc2[:], scale128[:], scale128[:], op=OP.mult)
        gs = pool.tile([128, 1], f32)
        nc.vector.tensor_tensor(gs[:], gamma_ap, scale128[:], op=OP.mult)
        bb1 = pool.tile([128, 1], f32)
        nc.vector.tensor_tensor(bb1[:], beta_ap, b1_ap, op=OP.add)

        # ---------------- stats: local reduce + AllGather ---------------
        loc = pool.tile([128, 2], f32)
        nc.vector.tensor_reduce(loc[:, 0:1], sums_t[:], axis=mybir.AxisListType.X,
                                op=OP.add)
        nc.vector.tensor_reduce(loc[:, 1:2], ssqs_t[:], axis=mybir.AxisListType.X,
                                op=OP.add)
        # fold partition halves (per-channel over all 4 local imgs),
        # replicated on both halves
        lsw = pool.tile([128, 2], f32)
        nc.sync.dma_start(lsw[0:64, :], loc[64:128, :])
        nc.sync.dma_start(lsw[64:128, :], loc[0:64, :])
        ssq = pool.tile([128, 2], f32)
        nc.vector.tensor_tensor(ssq[:], loc[:], lsw[:], op=OP.add)

        if sync_bn:
            ag_in = dram.tile([128, 2], f32)
            ag_out = dram.tile([128 * N_CORES, 2], f32)
            nc.sync.dma_start(ag_in[:], ssq[:])
            nc.gpsimd.collective_compute(
                "AllGather", OP.bypass, ins=[ag_in.opt()], outs=[ag_out.opt()],
                replica_groups=[list(range(N_CORES))])
            gath = pool.tile([128, 2, N_CORES], f32)
            nc.sync.dma_start(gath[:],
                              ag_out[:].rearrange("(k p) s -> p s k", k=N_CORES))
            tot = pool.tile([128, 2], f32)
            nc.vector.tensor_reduce(tot[:], gath[:], axis=mybir.AxisListType.X,
                                    op=OP.add)
            n_total = float(NG)
        else:
            tot = ssq
            n_total = float(IMGS * H * W)

        # ---------------- A, B computation (128-wide) ----------------
        me = pool.tile([128, 2], f32)   # (mean, E[x^2])
        nc.vector.tensor_scalar(me[:], tot[:], 1.0 / n_total, None, op0=OP.mult)
        mean_g = me[:, 0:1]
        var_r = pool.tile([128, 1], f32)
        nc.vector.tensor_tensor(var_r[:], mean_g, mean_g, op=OP.mult)
        nc.vector.tensor_tensor(var_r[:], me[:, 1:2], var_r[:], op=OP.subtract)
        vpe = pool.tile([128, 1], f32)
        nc.vector.tensor_scalar(vpe[:], var_r[:], sc2[:], BN_EPS,
                                op0=OP.mult, op1=OP.add)
        sq = pool.tile([128, 1], f32)
        nc.scalar.activation(sq[:], vpe[:], AF.Sqrt)
        r0_t = pool.tile([128, 1], f32)
        nc.vector.reciprocal(r0_t[:], sq[:])
        ab = pool.tile([128, 2], f32)
        nc.vector.tensor_tensor(ab[:, 0:1], r0_t[:], gs[:], op=OP.mult)
        mA = pool.tile([128, 1], f32)
        nc.vector.tensor_tensor(mA[:], mean_g, ab[:, 0:1], op=OP.mult)
        nc.vector.tensor_tensor(ab[:, 1:2], bb1[:], mA[:], op=OP.subtract)
        A_ap = ab[:, 0:1]
        B_ap = ab[:, 1:2]

        # ---------------- epilogue: A*raw + x, prelu, store -------------
        out_v = out_d.ap().rearrange("i c h w -> (i c) h w")
        ot = None
        for r in range(NCHUNK):
            if r % GRP == 0:
                ot = outp.tile([128, 2, GRP * RPC, W], f32, tag="ot")
            g = r % GRP
            rv = raw_t[:, :, r, :].rearrange("p s (r c) -> p s r c", r=4)
            xv = x_sb[:, :, r * RPC:(r + 1) * RPC, :]
            pe = psum.tile([128, 2, 512], f32, tag="cv")
            tv = pe[:, :, 0:448].rearrange("p s (r c) -> p s r c", r=4)
            nc.vector.scalar_tensor_tensor(tv, rv, A_ap, xv,
                                           op0=OP.mult, op1=OP.add)
            ov = ot[:, :, g * RPC:(g + 1) * RPC, :]
            nc.scalar.activation(ov, tv, AF.Prelu, bias=B_ap, scale=1.0,
                                 alpha=alpha_ap)
            if with_b2:
                nc.vector.tensor_scalar(ov, ov, b2_ap, None, op0=OP.add)
            if g == GRP - 1:
                r0 = (r - GRP + 1) * RPC
                eng = nc.sync
                for s in range(2):
                    dst = out_v[128 * s:128 * (s + 1), r0:r0 + GRP * RPC, :]
                    eng.dma_start(dst, ot[:, s, :, :])

    nc.compile()
    return nc


_CACHE = {}


def _get_program(with_b0: bool, with_b2: bool, sync_bn: bool):
    key = (with_b0, with_b2, sync_bn)
    if key not in _CACHE:
        _CACHE[key] = build_program(with_b0, with_b2, sync_bn)
    return _CACHE[key]


def run_sharded(inputs: dict, trace: bool = False, tmpdir=None):
    """Shard, run on 8 cores, gather. Returns (out, BassKernelResults)."""
    x = np.ascontiguousarray(np.asarray(inputs["x"], dtype=np.float32))
    w = np.ascontiguousarray(np.asarray(inputs["w"], dtype=np.float32))
    b0 = np.ascontiguousarray(np.asarray(inputs["b0"], dtype=np.float32))
    gamma = np.ascontiguousarray(np.asarray(inputs["gamma"], dtype=np.float32))
    beta = np.ascontiguousarray(np.asarray(inputs["beta"], dtype=np.float32))
    b1 = np.ascontiguousarray(np.asarray(inputs["b1"], dtype=np.float32))
    alpha = np.ascontiguousarray(np.asarray(inputs["alpha"], dtype=np.float32))
    b2 = np.ascontiguousarray(np.asarray(inputs["b2"], dtype=np.float32))
    with_b0 = bool(np.any(b0 != 0.0))
    with_b2 = bool(np.any(b2 != 0.0))
    sync_bn = os.environ.get("BK_NOSYNC", "0") != "1"
    nc = _get_program(with_b0, with_b2, sync_bn)

    # host-marshalled transposed weight layout wt[i, t, o]
    wt = np.ascontiguousarray(w.transpose(1, 2, 3, 0).reshape(C, 9, C))
    in_maps = []
    for k in range(N_CORES):
        in_maps.append({
            "x": np.ascontiguousarray(x[IMGS * k:IMGS * (k + 1)]),
            "w": w, "wt": wt, "b0": b0, "gamma": gamma, "beta": beta,
            "b1": b1, "alpha": alpha, "b2": b2,
        })
    res = run_bass_kernel_spmd(nc, in_maps, list(range(N_CORES)),
                               trace=trace, tmpdir=tmpdir)
    out = np.concatenate([res.results[k]["out"] for k in range(N_CORES)], axis=0)
    return out, res


def kernel(**inputs) -> np.ndarray:
    out, _ = run_sharded(inputs, trace=False)
    return out


# revision 27
# speedup vs baseline: 1.1051x; 1.1051x over previous
"""Trainium2 Bass kernel for nn_BasicBlock (binary-activation conv block).

Reference forward (per element):
    act  = sign(x + b0)                      # {-1, 0, +1}
    bw   = scale_c * sign(w),  scale_c = mean|w| over (ci,kh,kw)
    raw  = conv3x3(act, sign(w))             # exact small integers
    y    = (scale*raw - mu) * rsqrt(var + eps) * gamma + beta + x + b1
    out  = prelu(y, alpha) + b2
with BN stats (mu, var) over the FULL batch (sync-BN across cores).

Strategy (8 NeuronCores, batch-sharded 4 imgs/core):
  - act/weights are +-1 in fp8e4 -> matmuls with fp32 PSUM accumulation are
    EXACT.  conv = 9 shifted matmuls (K=64, M=64); FOUR streams run
    concurrently in the 128x128 PE array quadrants via tile_position.
  - SINGLE conv pass: each chunk's psum is copied to SBUF as fp16 (raw
    values are integers |raw| <= 576 -> exact in fp16).  The copy also
    accumulates the per-partition SUM (accum_out); GpSimd computes the
    per-partition SUMSQ from the fp16 copy.  No conv recompute, no
    bn_stats bottleneck on DVE.
  - sync-BN: a tiny warm-up AllGather at t~0 absorbs core-launch skew and
    CC stream spin-up; the real stats exchange is one small AllGather of
    per-channel (sum, sumsq) + local reduce.
  - epilogue (post-stats, tensor-free):
        t   = A*raw + x          (scalar_tensor_tensor; DVE and GpSimd
                                  alternate chunks)
        out = Prelu(t + B)       (one ACT pass, per-channel alpha)
    where A = gamma*scale*rsqrt(var+eps), B = beta + b1 - mu*A.
  - x streamed in 8 depth-2-chained DMA blocks so act bands become
    available early while the tail still streams at full HBM bandwidth.

kernel(**inputs) takes FULL inputs, shards, runs SPMD on cores 0-7, gathers.
"""
import os
import numpy as np
from contextlib import ExitStack

from concourse import bacc, mybir, tile
from concourse.tile_rust import add_dep_helper
from concourse.bass_utils import run_bass_kernel_spmd

# ---------------- problem constants (hardcoded per spec) ----------------
N_CORES = 8
IMGS = 4          # images per core
C = 64            # channels
H = W = 112
HP = WP = 114     # zero-padded act dims
BN_EPS = 1e-5
NG = 32 * H * W   # global BN count per channel

f32 = mybir.dt.float32
f16 = mybir.dt.float16
fp8 = mybir.dt.float8e4

RPC = 4            # output rows per psum bank
NCHUNK = H // RPC  # 28 row-chunks
GRP = 2            # chunks per staged output tile

# x stream blocks: 8 DMAs total (2 per block) so the 8 DMA-completion
# semaphore lanes never throttle the stream; a small first block gets the
# first act band ready early
XBLK = [(0, 15), (15, 29), (29, 57), (57, 85), (85, 112)]


def build_program(with_b0: bool, with_b2: bool, sync_bn: bool):
    nc = bacc.Bacc("TRN2", target_bir_lowering=False, debug=False,
                   num_devices=N_CORES)

    x_d = nc.dram_tensor("x", [IMGS, C, H, W], f32, kind="ExternalInput")
    b0_d = nc.dram_tensor("b0", [1, C, 1, 1], f32, kind="ExternalInput")
    w_d = nc.dram_tensor("w", [C, C, 3, 3], f32, kind="ExternalInput")
    gamma_d = nc.dram_tensor("gamma", [C], f32, kind="ExternalInput")
    beta_d = nc.dram_tensor("beta", [C], f32, kind="ExternalInput")
    b1_d = nc.dram_tensor("b1", [1, C, 1, 1], f32, kind="ExternalInput")
    alpha_d = nc.dram_tensor("alpha", [C], f32, kind="ExternalInput")
    b2_d = nc.dram_tensor("b2", [1, C, 1, 1], f32, kind="ExternalInput")
    # host-marshalled transposed weights: wt[i, t, o] = w[o, i, kh, kw]
    wt_d = nc.dram_tensor("wt", [C, 9, C], f32, kind="ExternalInput")
    out_d = nc.dram_tensor("out", [IMGS, C, H, W], f32, kind="ExternalOutput")

    AF = mybir.ActivationFunctionType
    OP = mybir.AluOpType

    with tile.TileContext(nc) as tc, ExitStack() as ctx:
        pool = ctx.enter_context(tc.tile_pool(name="sbuf", bufs=1))
        actp = ctx.enter_context(tc.tile_pool(name="actp", bufs=3))
        stgp = ctx.enter_context(tc.tile_pool(name="stgp", bufs=2))
        outp = ctx.enter_context(tc.tile_pool(name="outp", bufs=3))
        psum = ctx.enter_context(
            tc.tile_pool(name="psum", bufs=4, space="PSUM"))
        dram = ctx.enter_context(tc.tile_pool(name="dram", bufs=1, space="DRAM"))

        # collective warm-up: absorbs core-launch skew + CC stream spin-up
        warm_sb = pool.tile([8, 4], f32)
        nc.gpsimd.memset(warm_sb[:], 0.0)
        warm_in = dram.tile([8, 4], f32)
        warm_out = dram.tile([64, 4], f32)
        nc.scalar.dma_start(warm_in[:], warm_sb[:])
        nc.gpsimd.collective_compute(
            "AllGather", OP.bypass, ins=[warm_in.opt()], outs=[warm_out.opt()],
            replica_groups=[list(range(N_CORES))])

        # ------------- transposed weights (gpsimd queue, ahead of x) ----
        wt_f = pool.tile([64, 9, 64], f32)
        nc.gpsimd.dma_start(wt_f[:], wt_d.ap().rearrange("i t o -> i (t o)")
                            .rearrange("i (t o) -> i t o", t=9))

        def load_params():
            """Small params; scalar-queue DMAs + DVE math.  Issued late
            (before stats) unless sign needs b0."""
            par = pool.tile([64, 6], f32)
            nc.sync.dma_start(par[:, 0:1], b0_d.ap().rearrange("a c e f -> (a c) (e f)"))
            nc.sync.dma_start(par[:, 1:2], gamma_d.ap().rearrange("c -> c ()"))
            nc.sync.dma_start(par[:, 2:3], beta_d.ap().rearrange("c -> c ()"))
            nc.sync.dma_start(par[:, 3:4], b1_d.ap().rearrange("a c e f -> (a c) (e f)"))
            nc.sync.dma_start(par[:, 4:5], alpha_d.ap().rearrange("c -> c ()"))
            nc.sync.dma_start(par[:, 5:6], b2_d.ap().rearrange("a c e f -> (a c) (e f)"))
            rep = pool.tile([128, 6], f32)
            nc.vector.tensor_copy(rep[0:64, :], par[:])
            nc.gpsimd.dma_start(rep[64:128, :], rep[0:64, :])
            return rep

        # ---------------- x load: 8 blocks, depth-2 chained -------------
        x_sb = pool.tile([128, 2, H, W], f32)
        x_v = x_d.ap().rearrange("i c h w -> (i c) h w")
        blk_dmas = []
        for k, (r0, r1) in enumerate(XBLK):
            cur = []
            for s in range(2):
                src = x_v[128 * s:128 * (s + 1), r0:r1, :]
                ins = nc.sync.dma_start(x_sb[:, s, r0:r1, :], src)
                cur.append(ins)
            blk_dmas.append(cur)

        # sgn of transposed weights -> fp8 (+-1 exact); conv needs only this
        with tc.high_priority():
            nc.vector.tensor_scalar(wt_f[:], wt_f[:], 0.0, None, op0=OP.is_gt)
            w_taps = pool.tile([128, 9, 64], fp8)
            nc.vector.tensor_scalar(w_taps[0:64, :, :], wt_f[:], 2.0, -1.0,
                                    op0=OP.mult, op1=OP.add)
            nc.gpsimd.dma_start(w_taps[64:128, :, :], w_taps[0:64, :, :])
        # |w| scale (only needed pre-stats; replicate deferred to the
        # late-params section so it never blocks the gpsimd act swaps)
        w_sb = pool.tile([64, 576], f32)
        nc.scalar.dma_start(w_sb[:], w_d.ap().rearrange("o i kh kw -> o (i kh kw)"))
        scale128 = pool.tile([128, 1], f32)
        nc.vector.tensor_reduce(scale128[0:64, :], w_sb[:], axis=mybir.AxisListType.X,
                                op=OP.add, apply_absolute_value=True)
        nc.vector.tensor_scalar(scale128[0:64, :], scale128[0:64, :], 1.0 / 576.0,
                                None, op0=OP.mult)

        rep = load_params() if with_b0 else None
        if rep is not None:
            b0_ap = rep[:, 0:1]

        # ---------------- raw / stats storage ----------------
        raw_t = pool.tile([128, 2, NCHUNK, 448], f16)
        sums_t = pool.tile([128, NCHUNK], f32)
        ssqs_t = pool.tile([128, NCHUNK], f32)
        junk = pool.tile([128, 2, 448], f32)
        junk2 = pool.tile([128, 2, 448], f32)


        # 4-quadrant interleave order per tap:
        #   (slot, act_half_base, tile_position, psum_base)
        QORDER = [
            (1, 0, (0, 64), 64),    # img3
            (0, 0, (0, 0), 0),      # img0
            (0, 64, (64, 64), 64),  # img1
            (1, 64, (64, 0), 0),    # img2
        ]

        sign_kw = dict(bias=rep[:, 0:1]) if with_b0 else {}

        def make_band(b):
            """Produce act band b: sign of padded rows [28b, 28b+30) in two
            units; slot1 half-swapped via a rotating stage tile."""
            ctx2 = ExitStack()
            ctx2.enter_context(tc.high_priority())
            ab_t = actp.tile([128, 2, 30, WP], fp8, tag="act")
            stg = stgp.tile([128, 30, WP], fp8, tag="stg")
            nc.gpsimd.memset(ab_t[:, 0, :, 0:WP:113], 0.0)
            nc.gpsimd.memset(stg[:, :, 0:WP:113], 0.0)
            if b == 0:
                nc.gpsimd.memset(ab_t[:, :, 0:1, :], 0.0)
            if b == 3:
                nc.gpsimd.memset(ab_t[:, :, 29:30, :], 0.0)
            lo = max(1, 28 * b)
            hi = min(113, 28 * b + 30)
            for (a0, a1) in ((lo, 28 * b + 16), (28 * b + 16, hi)):
                nr = a1 - a0
                l0 = a0 - 28 * b
                xr = a0 - 1
                nc.scalar.activation(ab_t[:, 0, l0:l0 + nr, 1:113],
                                     x_sb[:, 0, xr:xr + nr, :], AF.Sign,
                                     **sign_kw)
                nc.scalar.activation(stg[:, l0:l0 + nr, 1:113],
                                     x_sb[:, 1, xr:xr + nr, :], AF.Sign,
                                     **sign_kw)
                nc.scalar.dma_start(ab_t[64:128, 1, l0:l0 + nr, :],
                                    stg[0:64, l0:l0 + nr, :])
                nc.scalar.dma_start(ab_t[0:64, 1, l0:l0 + nr, :],
                                    stg[64:128, l0:l0 + nr, :])
            ctx2.close()
            return ab_t

        def conv_pair(pt, band, lrow):
            """9-tap binary conv for one row-chunk of BOTH slots into one
            [128, 2, 512] psum tile (bank s = slot s, cols 0:448 used),
            matmuls interleaved across all 4 PE quadrants."""
            for t in range(9):
                ky, kx = divmod(t, 3)
                for (s, ab, tp, pb) in QORDER:
                    rhs = band[ab:ab + 64, s, lrow + ky:lrow + ky + 4, kx:kx + 112]
                    dst = pt[pb:pb + 64, s, 0:448].rearrange(
                        "p (r c) -> p r c", r=4)
                    nc.tensor.matmul(
                        dst, w_taps[ab:ab + 64, t, :], rhs,
                        start=(t == 0), stop=(t == 8), tile_position=tp)

        # ------------- conv pass: psum -> fp16 raw + sum/sumsq ----------
        bands = [make_band(0)]
        for r in range(NCHUNK):
            b, l = divmod(r, 7)
            if l == 0 and b + 1 < 4:
                bands.append(make_band(b + 1))
            band, lrow = bands[b], 4 * l
            pt = psum.tile([128, 2, 512], f32, tag="cv")
            conv_pair(pt, band, lrow)
            # exact fp16 copy of the integer conv output; accumulates the
            # per-partition sum as a side effect
            nc.vector.tensor_scalar(raw_t[:, :, r, :], pt[:, :, 0:448],
                                    1.0, 0.0, op0=OP.mult, op1=OP.add,
                                    accum_out=sums_t[:, r:r + 1])
            # sumsq, odd chunks: fused DVE stt + accum (paces with the copy)
            if r % 2 == 1:
                nc.vector.scalar_tensor_tensor(junk[:], raw_t[:, :, r, :], 1.0,
                                               raw_t[:, :, r, :], op0=OP.mult,
                                               op1=OP.mult,
                                               accum_out=ssqs_t[:, r:r + 1])

        # sumsq, even chunks: ACT Square + accum, issued AFTER all sign work
        # so band production is never head-of-line blocked on the ACT queue
        for r in range(0, NCHUNK, 2):
            nc.scalar.activation(junk2[:], raw_t[:, :, r, :], AF.Square,
                                 accum_out=ssqs_t[:, r:r + 1])

        # ---------------- late params + stats-independent math ----------
        if rep is None:
            rep = load_params()
        nc.gpsimd.dma_start(scale128[64:128, :], scale128[0:64, :])
        gamma_ap = rep[:, 1:2]
        beta_ap = rep[:, 2:3]
        b1_ap = rep[:, 3:4]
        alpha_ap = rep[:, 4:5]
        b2_ap = rep[:, 5:6]
        sc2 = pool.tile([128, 1], f32)
        nc.vector.tensor_tensor(sc2[:], scale128[:], scale128[:], op=OP.mult)
        gs = pool.tile([128, 1], f32)
        nc.vector.tensor_tensor(gs[:], gamma_ap, scale128[:], op=OP.mult)
        bb1 = pool.tile([128, 1], f32)
        nc.vector.tensor_tensor(bb1[:], beta_ap, b1_ap, op=OP.add)

        # ---------------- stats: local reduce + AllGather ---------------
        loc = pool.tile([128, 2], f32)
        nc.vector.tensor_reduce(loc[:, 0:1], sums_t[:], axis=mybir.AxisListType.X,
                                op=OP.add)
        nc.vector.tensor_reduce(loc[:, 1:2], ssqs_t[:], axis=mybir.AxisListType.X,
                                op=OP.add)
        # fold partition halves (per-channel over all 4 local imgs),
        # replicated on both halves
        lsw = pool.tile([128, 2], f32)
        nc.sync.dma_start(lsw[0:64, :], loc[64:128, :])
        nc.sync.dma_start(lsw[64:128, :], loc[0:64, :])
        ssq = pool.tile([128, 2], f32)
        nc.vector.tensor_tensor(ssq[:], loc[:], lsw[:], op=OP.add)

        if sync_bn:
            ag_in = dram.tile([128, 2], f32)
            ag_out = dram.tile([128 * N_CORES, 2], f32)
            nc.sync.dma_start(ag_in[:], ssq[:])
            nc.gpsimd.collective_compute(
                "AllGather", OP.bypass, ins=[ag_in.opt()], outs=[ag_out.opt()],
                replica_groups=[list(range(N_CORES))])
            gath = pool.tile([128, 2, N_CORES], f32)
            nc.sync.dma_start(gath[:],
                              ag_out[:].rearrange("(k p) s -> p s k", k=N_CORES))
            tot = pool.tile([128, 2], f32)
            nc.vector.tensor_reduce(tot[:], gath[:], axis=mybir.AxisListType.X,
                                    op=OP.add)
            n_total = float(NG)
        else:
            tot = ssq
            n_total = float(IMGS * H * W)

        # ---------------- A, B computation (128-wide) ----------------
        me = pool.tile([128, 2], f32)   # (mean, E[x^2])
        nc.vector.tensor_scalar(me[:], tot[:], 1.0 / n_total, None, op0=OP.mult)
        mean_g = me[:, 0:1]
        var_r = pool.tile([128, 1], f32)
        nc.vector.tensor_tensor(var_r[:], mean_g, mean_g, op=OP.mult)
        nc.vector.tensor_tensor(var_r[:], me[:, 1:2], var_r[:], op=OP.subtract)
        vpe = pool.tile([128, 1], f32)
        nc.vector.tensor_scalar(vpe[:], var_r[:], sc2[:], BN_EPS,
                                op0=OP.mult, op1=OP.add)
        sq = pool.tile([128, 1], f32)
        nc.scalar.activation(sq[:], vpe[:], AF.Sqrt)
        r0_t = pool.tile([128, 1], f32)
        nc.vector.reciprocal(r0_t[:], sq[:])
        ab = pool.tile([128, 2], f32)
        nc.vector.tensor_tensor(ab[:, 0:1], r0_t[:], gs[:], op=OP.mult)
        mA = pool.tile([128, 1], f32)
        nc.vector.tensor_tensor(mA[:], mean_g, ab[:, 0:1], op=OP.mult)
        nc.vector.tensor_tensor(ab[:, 1:2], bb1[:], mA[:], op=OP.subtract)
        A_ap = ab[:, 0:1]
        B_ap = ab[:, 1:2]

        # ---------------- epilogue: A*raw + x, prelu, store -------------
        out_v = out_d.ap().rearrange("i c h w -> (i c) h w")
        ot = None
        for r in range(NCHUNK):
            if r % GRP == 0:
                ot = outp.tile([128, 2, GRP * RPC, W], f32, tag="ot")
            g = r % GRP
            rv = raw_t[:, :, r, :].rearrange("p s (r c) -> p s r c", r=4)
            xv = x_sb[:, :, r * RPC:(r + 1) * RPC, :]
            pe = psum.tile([128, 2, 512], f32, tag="cv")
            tv = pe[:, :, 0:448].rearrange("p s (r c) -> p s r c", r=4)
            nc.vector.scalar_tensor_tensor(tv, rv, A_ap, xv,
                                           op0=OP.mult, op1=OP.add)
            ov = ot[:, :, g * RPC:(g + 1) * RPC, :]
            nc.scalar.activation(ov, tv, AF.Prelu, bias=B_ap, scale=1.0,
                                 alpha=alpha_ap)
            if with_b2:
                nc.vector.tensor_scalar(ov, ov, b2_ap, None, op0=OP.add)
            if g == GRP - 1:
                r0 = (r - GRP + 1) * RPC
                eng = nc.sync
                for s in range(2):
                    dst = out_v[128 * s:128 * (s + 1), r0:r0 + GRP * RPC, :]
                    eng.dma_start(dst, ot[:, s, :, :])

    nc.compile()
    return nc


_CACHE = {}


def _get_program(with_b0: bool, with_b2: bool, sync_bn: bool):
    key = (with_b0, with_b2, sync_bn)
    if key not in _CACHE:
        _CACHE[key] = build_program(with_b0, with_b2, sync_bn)
    return _CACHE[key]


def run_sharded(inputs: dict, trace: bool = False, tmpdir=None):
    """Shard, run on 8 cores, gather. Returns (out, BassKernelResults)."""
    x = np.ascontiguousarray(np.asarray(inputs["x"], dtype=np.float32))
    w = np.ascontiguousarray(np.asarray(inputs["w"], dtype=np.float32))
    b0 = np.ascontiguousarray(np.asarray(inputs["b0"], dtype=np.float32))
    gamma = np.ascontiguousarray(np.asarray(inputs["gamma"], dtype=np.float32))
    beta = np.ascontiguousarray(np.asarray(inputs["beta"], dtype=np.float32))
    b1 = np.ascontiguousarray(np.asarray(inputs["b1"], dtype=np.float32))
    alpha = np.ascontiguousarray(np.asarray(inputs["alpha"], dtype=np.float32))
    b2 = np.ascontiguousarray(np.asarray(inputs["b2"], dtype=np.float32))
    with_b0 = bool(np.any(b0 != 0.0))
    with_b2 = bool(np.any(b2 != 0.0))
    sync_bn = os.environ.get("BK_NOSYNC", "0") != "1"
    nc = _get_program(with_b0, with_b2, sync_bn)

    # host-marshalled transposed weight layout wt[i, t, o]
    wt = np.ascontiguousarray(w.transpose(1, 2, 3, 0).reshape(C, 9, C))
    in_maps = []
    for k in range(N_CORES):
        in_maps.append({
            "x": np.ascontiguousarray(x[IMGS * k:IMGS * (k + 1)]),
            "w": w, "wt": wt, "b0": b0, "gamma": gamma, "beta": beta,
            "b1": b1, "alpha": alpha, "b2": b2,
        })
    res = run_bass_kernel_spmd(nc, in_maps, list(range(N_CORES)),
                               trace=trace, tmpdir=tmpdir)
    out = np.concatenate([res.results[k]["out"] for k in range(N_CORES)], axis=0)
    return out, res


def kernel(**inputs) -> np.ndarray:
    out, _ = run_sharded(inputs, trace=False)
    return out
